# revision 6
# baseline (speedup 1.0000x reference)
"""Trainium2 Bass kernel for nn_Combiner (retrieval_knn / DPC-KNN token merge).

Pipeline per batch element:
  residual token Conv1d(k=3) -> LayerNorm -> token_weight=exp(x@w+b)
  cdist -> 3NN density -> dist-to-higher-density -> top-2 centers ->
  argmin cluster assign -> weighted segment-sum merge.

Data-parallel over batch: 16 batches -> 8 NeuronCores, 2 each.
All matmuls run in native fp32 (exact f32 products); distance order
statistics are computed in the d2 domain (monotone-equivalent), sqrt is
applied only to tiny per-token stats with one Newton polish step.
"""
import sys, os
sys.path.insert(0, '/opt/trn_rl_repo')
import numpy as np
import concourse.bass as bass
import concourse.mybir as mybir
import concourse.tile as tile
import concourse.bacc as bacc
from concourse.alu_op_type import AluOpType
from concourse.bass import IndirectOffsetOnAxis
from concourse.bass_utils import run_bass_kernel_spmd

F32 = mybir.dt.float32
I32 = mybir.dt.int32
AF = mybir.ActivationFunctionType
AX = mybir.AxisListType
OP = AluOpType

BIG = 1.0e9          # additive exclusion constant (d2 domain)
LN_EPS = 1e-5
P = 128


def _splits(total, maxn):
    n = (total + maxn - 1) // maxn
    base = total // n
    assert base * n == total
    return [base] * n


def build_program(n_cores, b_per_core, N, C):
    NBLK = N // P
    CCH = C // P
    JSP = _splits(N, 512)                              # gram col chunks
    HSP = _splits(C, 384 if C % 384 == 0 else 512)     # cout / merge splits
    INV_SQRT_C = float(np.float32(1.0) / np.float32(np.sqrt(np.float32(C))))
    INV_3 = float(np.float32(1.0) / np.float32(3.0))

    nc = bacc.Bacc("TRN2", target_bir_lowering=False, debug=False,
                   num_devices=n_cores)

    x_d = nc.dram_tensor("x", [b_per_core, N, C], F32, kind="ExternalInput")
    wt_d = nc.dram_tensor("wt", [C, 3 * C], F32, kind="ExternalInput")
    gam_d = nc.dram_tensor("gam", [C], F32, kind="ExternalInput")
    bet_d = nc.dram_tensor("bet", [C], F32, kind="ExternalInput")
    sw_d = nc.dram_tensor("sw", [C], F32, kind="ExternalInput")
    sb_d = nc.dram_tensor("sb", [1], F32, kind="ExternalInput")
    noise_d = nc.dram_tensor("noise", [b_per_core, N], F32, kind="ExternalInput")
    ident_d = nc.dram_tensor("ident", [P, P], F32, kind="ExternalInput")
    identi_d = nc.dram_tensor("identi", [P, P], I32, kind="ExternalInput")
    iota_tc_d = nc.dram_tensor("iota_tc", [P, NBLK], F32, kind="ExternalInput")

    xm_o = nc.dram_tensor("xm", [b_per_core, 2, C], F32, kind="ExternalOutput")
    agg_o = nc.dram_tensor("agg", [b_per_core, N], F32, kind="ExternalOutput")
    idx_o = nc.dram_tensor("idx", [b_per_core, N], I32, kind="ExternalOutput")

    with tile.TileContext(nc) as tc:
        with tc.tile_pool(name="pconst", bufs=1) as pconst, \
             tc.tile_pool(name="pbig", bufs=1) as pbig, \
             tc.tile_pool(name="pw", bufs=6) as pw, \
             tc.tile_pool(name="prow", bufs=1) as prow, \
             tc.tile_pool(name="pmid", bufs=2) as pmid, \
             tc.tile_pool(name="pstat", bufs=2) as pstat, \
             tc.tile_pool(name="pps", bufs=1, space="PSUM") as pps, \
             tc.tile_pool(name="pps2", bufs=2, space="PSUM") as pps2, \
             tc.tile_pool(name="pdram", bufs=2, space="DRAM") as pdram:

            # ---------- constants ----------
            ident = pconst.tile([P, P], F32, tag="ident")
            nc.sync.dma_start(ident[:], ident_d[:])
            identi = pconst.tile([P, P], I32, tag="identi")
            nc.sync.dma_start(identi[:], identi_d[:])
            bigtile = pconst.tile([P, P], F32, tag="bigtile")
            nc.vector.memset(bigtile[:], BIG)
            iota_tc = pconst.tile([P, NBLK], F32, tag="iota_tc")
            nc.sync.dma_start(iota_tc[:], iota_tc_d[:])
            zero_tm = pconst.tile([P, NBLK], F32, tag="zero_tm")
            nc.vector.memset(zero_tm[:], 0.0)
            one_tm = pconst.tile([P, NBLK], F32, tag="one_tm")
            nc.vector.memset(one_tm[:], 1.0)
            ones_col = pconst.tile([P, 1], F32, tag="ones_col")
            nc.vector.memset(ones_col[:], 1.0)
            gam_bc = pconst.tile([P, C], F32, tag="gam_bc")
            nc.sync.dma_start(gam_bc[:], gam_d[:].unsqueeze(0).to_broadcast((P, C)))
            bet_bc = pconst.tile([P, C], F32, tag="bet_bc")
            nc.sync.dma_start(bet_bc[:], bet_d[:].unsqueeze(0).to_broadcast((P, C)))
            sw_t = pconst.tile([P, CCH], F32, tag="sw_t")
            nc.sync.dma_start(sw_t[:], sw_d[:].rearrange("(c p) -> p c", p=P))
            sb_t = pconst.tile([1, 1], F32, tag="sb_t")
            nc.sync.dma_start(sb_t[:], sb_d[:].unsqueeze(0))
            # ---------- helpers ----------
            def bcast_scalar(src11, name):
                """[1,1] sbuf -> [128,1] broadcast tile (via DRAM)."""
                d = pdram.tile([1], F32, tag=f"bs_{name}")
                nc.sync.dma_start(d[:], src11[:, 0])
                bc = pstat.tile([P, 1], F32, tag=f"bsb_{name}")
                nc.sync.dma_start(bc[:], d[:].unsqueeze(0).to_broadcast((P, 1)))
                return bc

            def glob_reduce(val_col, op, name):
                """[128,1] -> scalar over partitions: returns ([1,1], [128,1]bc)."""
                d = pdram.tile([P], F32, tag=f"gr_{name}")
                nc.sync.dma_start(d[:], val_col[:, 0])
                row = pstat.tile([1, P], F32, tag=f"grrow_{name}")
                nc.sync.dma_start(row[:], d[:].unsqueeze(0))
                sc = pstat.tile([1, 1], F32, tag=f"grsc_{name}")
                nc.vector.tensor_reduce(sc[:], row[:], AX.X, op)
                bc = bcast_scalar(sc, f"gr_{name}")
                return sc, bc

            def sqrt_nr(x_ap, out_ap, tag, shape):
                """out = sqrt(x), ACT seed + one Newton step. x > 0 required."""
                s0 = pstat.tile(shape, F32, tag=f"nr_s0_{tag}")
                nc.scalar.activation(s0[:], x_ap, AF.Sqrt, bias=0.0, scale=1.0)
                r0 = pstat.tile(shape, F32, tag=f"nr_r0_{tag}")
                nc.vector.reciprocal(r0[:], s0[:])
                t0 = pstat.tile(shape, F32, tag=f"nr_t0_{tag}")
                nc.vector.tensor_tensor(t0[:], x_ap, r0[:], OP.mult)
                nc.vector.tensor_tensor(t0[:], s0[:], t0[:], OP.add)
                nc.vector.tensor_scalar(out_ap, t0[:], 0.5, None, op0=OP.mult)

            # =========================================================
            for b in range(b_per_core):
                wtiles = []
                for c in range(CCH):
                    w_c = pw.tile([P, 3 * C], F32, tag="wslot")
                    nc.sync.dma_start(w_c[:], wt_d[c * P:(c + 1) * P, :])
                    wtiles.append(w_c)
                xTin = []
                for c in range(CCH):
                    t_ = pbig.tile([P, N + 2], F32, tag=f"xtin{c}")
                    nc.vector.memset(t_[:, 0:1], 0.0)
                    nc.vector.memset(t_[:, N + 1:N + 2], 0.0)
                    xTin.append(t_)
                stats_sq = pstat.tile([P, NBLK], F32, tag="stats_sq")
                stats_n1 = pstat.tile([P, NBLK], F32, tag="stats_n1")
                stats_n2 = pstat.tile([P, NBLK], F32, tag="stats_n2")
                stats_rmax = pstat.tile([P, NBLK], F32, tag="stats_rmax")
                stats_dmin = pstat.tile([P, NBLK], F32, tag="stats_dmin")

                xs_dd = pdram.tile([N, C], F32, tag="xs_dd")
                u_dd = pdram.tile([N, N], F32, tag="u_dd")
                sq_row_d = pdram.tile([N], F32, tag="sq_row_d")

                # ---- phase 1a: transpose x into xTin ----
                for blk in range(NBLK):
                    x_t = pmid.tile([P, C], F32, tag="x_t")
                    nc.sync.dma_start(x_t[:], x_d[b, blk * P:(blk + 1) * P, :])
                    for c in range(CCH):
                        ps_t = pps2.tile([P, P], F32, tag="ps_t")
                        nc.tensor.transpose(ps_t[:], x_t[:, c * P:(c + 1) * P], ident[:])
                        nc.scalar.copy(xTin[c][:, 1 + blk * P:1 + (blk + 1) * P], ps_t[:])

                # ---- phase 1b: conv + residual + LN -> xs, xsT ----
                for blk in range(NBLK):
                    ps_cv = []
                    for hi, h in enumerate(HSP):
                        p_h = pps.tile([P, h], F32, tag=f"ps_cv{hi}")
                        nmm = 0
                        for c in range(CCH):
                            for k in range(3):
                                nmm += 1
                                nc.tensor.matmul(
                                    p_h[:],
                                    xTin[c][:, blk * P + k: blk * P + k + P],
                                    wtiles[c][:, k * C + sum(HSP[:hi]):
                                              k * C + sum(HSP[:hi]) + h],
                                    start=(nmm == 1), stop=(nmm == 3 * CCH))
                        ps_cv.append(p_h)
                    x_t = pmid.tile([P, C], F32, tag="x_t")
                    nc.sync.dma_start(x_t[:], x_d[b, blk * P:(blk + 1) * P, :])
                    xnew = pmid.tile([P, C], F32, tag="xnew")
                    off = 0
                    for hi, h in enumerate(HSP):
                        nc.vector.tensor_tensor(xnew[:, off:off + h], ps_cv[hi][:],
                                                x_t[:, off:off + h], OP.add)
                        off += h
                    mun = pstat.tile([P, 1], F32, tag="mun")
                    nc.vector.tensor_reduce(mun[:], xnew[:], AX.X, OP.add)
                    nc.vector.tensor_scalar(mun[:], mun[:], -1.0 / C, None, op0=OP.mult)
                    scr_c = pmid.tile([P, C], F32, tag="scr_c")
                    ssq = pstat.tile([P, 1], F32, tag="ssq")
                    nc.scalar.activation(scr_c[:], xnew[:], AF.Square,
                                         bias=mun[:], scale=1.0, accum_out=ssq[:])
                    veps = pstat.tile([P, 1], F32, tag="veps")
                    nc.vector.tensor_scalar(veps[:], ssq[:], 1.0 / C, LN_EPS,
                                            op0=OP.mult, op1=OP.add)
                    sdev = pstat.tile([P, 1], F32, tag="sdev")
                    sqrt_nr(veps[:], sdev[:], "ln", [P, 1])
                    rstd = pstat.tile([P, 1], F32, tag="rstd")
                    nc.vector.reciprocal(rstd[:], sdev[:])
                    xs_t = pmid.tile([P, C], F32, tag="xs_t")
                    nc.vector.tensor_scalar(xs_t[:], xnew[:], mun[:], rstd[:],
                                            op0=OP.add, op1=OP.mult)
                    nc.vector.tensor_tensor(xs_t[:], xs_t[:], gam_bc[:], OP.mult)
                    nc.vector.tensor_tensor(xs_t[:], xs_t[:], bet_bc[:], OP.add)
                    nc.sync.dma_start(xs_dd[blk * P:(blk + 1) * P, :], xs_t[:])

                # ---- phase 1b2: transpose xs into xsT (reuses xTin slots) ----
                xsT = []
                for c in range(CCH):
                    t_ = pbig.tile([P, N], F32, tag=f"xtin{c}")
                    xsT.append(t_)
                for blk in range(NBLK):
                    xs_t = pmid.tile([P, C], F32, tag="xs_t")
                    nc.sync.dma_start(xs_t[:], xs_dd[blk * P:(blk + 1) * P, :])
                    for c in range(CCH):
                        ps_t = pps2.tile([P, P], F32, tag="ps_t")
                        nc.tensor.transpose(ps_t[:], xs_t[:, c * P:(c + 1) * P], ident[:])
                        nc.scalar.copy(xsT[c][:, blk * P:(blk + 1) * P], ps_t[:])

                # ---- phase 1c: token weights w = exp(score + b) ----
                w_rowt = prow.tile([1, N], F32, tag="rowscratch")
                joff = 0
                for jn in JSP:
                    ps_s = pps.tile([1, 512], F32, tag="ps_s")
                    for c in range(CCH):
                        nc.tensor.matmul(ps_s[:, :jn], sw_t[:, c:c + 1],
                                         xsT[c][:, joff:joff + jn],
                                         start=(c == 0), stop=(c == CCH - 1))
                    nc.scalar.activation(w_rowt[:, joff:joff + jn], ps_s[:, :jn],
                                         AF.Exp, bias=sb_t[:], scale=1.0)
                    joff += jn
                w_row_d = pdram.tile([N], F32, tag="w_row_d")
                nc.sync.dma_start(w_row_d[:].unsqueeze(0), w_rowt[0:1, :])
                w_tm = pstat.tile([P, NBLK], F32, tag="w_tm")
                nc.sync.dma_start(w_tm[:], w_row_d[:].rearrange("(q p) -> p q", p=P))

                # ---- phase 2a: diag blocks -> sq = g_ii ----
                for blk in range(NBLK):
                    ps_d = pps2.tile([P, P], F32, tag="ps_t")
                    for c in range(CCH):
                        nc.tensor.matmul(ps_d[:],
                                         xsT[c][:, blk * P:(blk + 1) * P],
                                         xsT[c][:, blk * P:(blk + 1) * P],
                                         start=(c == 0), stop=(c == CCH - 1))
                    scr_p = pmid.tile([P, P], F32, tag="scr_p")
                    nc.vector.tensor_tensor(scr_p[:], ps_d[:], ident[:], OP.mult)
                    nc.vector.tensor_reduce(stats_sq[:, blk:blk + 1], scr_p[:],
                                            AX.X, OP.add)
                nc.sync.dma_start(sq_row_d[:].rearrange("(q p) -> p q", p=P),
                                  stats_sq[:])
                sq_bc = pbig.tile([P, N], F32, tag="bcrow")
                nc.sync.dma_start(sq_bc[:], sq_row_d[:].unsqueeze(0).to_broadcast((P, N)))

                # ---- phase 2b: gram -> u = -2g + sq_j ; NN stats ----
                for blk in range(NBLK):
                    u_t = pw.tile([P, N], F32, tag="wslot")
                    joff = 0
                    for jn in JSP:
                        ps_g = pps2.tile([P, 512], F32, tag="ps_g")
                        for c in range(CCH):
                            nc.tensor.matmul(ps_g[:, :jn],
                                             xsT[c][:, blk * P:(blk + 1) * P],
                                             xsT[c][:, joff:joff + jn],
                                             start=(c == 0), stop=(c == CCH - 1))
                        nc.vector.scalar_tensor_tensor(
                            u_t[:, joff:joff + jn], ps_g[:, :jn], -2.0,
                            sq_bc[:, joff:joff + jn], op0=OP.mult, op1=OP.add)
                        joff += jn
                    nc.vector.tensor_reduce(stats_rmax[:, blk:blk + 1], u_t[:],
                                            AX.X, OP.max)
                    nc.vector.copy_predicated(u_t[:, blk * P:(blk + 1) * P],
                                              identi[:], bigtile[:])
                    nc.vector.tensor_reduce(stats_n1[:, blk:blk + 1], u_t[:],
                                            AX.X, OP.min)
                    eqb = pw.tile([P, N], F32, tag="wslot")
                    nc.vector.tensor_scalar(eqb[:], u_t[:],
                                            stats_n1[:, blk:blk + 1], BIG,
                                            op0=OP.is_equal, op1=OP.mult)
                    nc.vector.tensor_tensor(eqb[:], u_t[:], eqb[:], OP.add)
                    nc.vector.tensor_reduce(stats_n2[:, blk:blk + 1], eqb[:],
                                            AX.X, OP.min)
                    nc.sync.dma_start(u_dd[blk * P:(blk + 1) * P, :], u_t[:])

                # ---- phase 3: density ----
                nn1 = pstat.tile([P, NBLK], F32, tag="nn1")
                nc.vector.tensor_tensor(nn1[:], stats_n1[:], stats_sq[:], OP.add)
                nc.vector.tensor_scalar(nn1[:], nn1[:], 0.0, None, op0=OP.max)
                nn2 = pstat.tile([P, NBLK], F32, tag="nn2")
                nc.vector.tensor_tensor(nn2[:], stats_n2[:], stats_sq[:], OP.add)
                nc.vector.tensor_scalar(nn2[:], nn2[:], 0.0, None, op0=OP.max)
                dn1 = pstat.tile([P, NBLK], F32, tag="dn1")
                sqrt_nr(nn1[:], dn1[:], "dn1", [P, NBLK])
                nc.vector.tensor_scalar(dn1[:], dn1[:], INV_SQRT_C, None, op0=OP.mult)
                dn2 = pstat.tile([P, NBLK], F32, tag="dn2")
                sqrt_nr(nn2[:], dn2[:], "dn2", [P, NBLK])
                nc.vector.tensor_scalar(dn2[:], dn2[:], INV_SQRT_C, None, op0=OP.mult)
                qq = pstat.tile([P, NBLK], F32, tag="qq")
                nc.vector.tensor_tensor(qq[:], dn1[:], dn1[:], OP.mult)
                q2 = pstat.tile([P, NBLK], F32, tag="q2")
                nc.vector.tensor_tensor(q2[:], dn2[:], dn2[:], OP.mult)
                nc.vector.tensor_tensor(qq[:], qq[:], q2[:], OP.add)
                dens = pstat.tile([P, NBLK], F32, tag="dens")
                nc.scalar.activation(dens[:], qq[:], AF.Exp, bias=0.0, scale=-INV_3)
                noise_t = pstat.tile([P, NBLK], F32, tag="noise_t")
                nc.sync.dma_start(noise_t[:],
                                  noise_d[b, :].rearrange("(q p) -> p q", p=P))
                nc.vector.scalar_tensor_tensor(dens[:], noise_t[:], 1e-6, dens[:],
                                               op0=OP.mult, op1=OP.add)
                dens_row_d = pdram.tile([N], F32, tag="dens_row_d")
                nc.sync.dma_start(dens_row_d[:].rearrange("(q p) -> p q", p=P),
                                  dens[:])
                dens_bc = pbig.tile([P, N], F32, tag="bcrow")
                nc.sync.dma_start(dens_bc[:],
                                  dens_row_d[:].unsqueeze(0).to_broadcast((P, N)))

                rmax_d2 = pstat.tile([P, NBLK], F32, tag="rmax_d2")
                nc.vector.tensor_tensor(rmax_d2[:], stats_rmax[:], stats_sq[:], OP.add)
                rmax_col = pstat.tile([P, 1], F32, tag="rmax_col")
                nc.vector.tensor_reduce(rmax_col[:], rmax_d2[:], AX.X, OP.max)
                _, dmax_bc = glob_reduce(rmax_col, OP.max, "dmax")

                # ---- phase 4: min dist to higher-density point ----
                for blk in range(NBLK):
                    u_t = pw.tile([P, N], F32, tag="wslot")
                    nc.sync.dma_start(u_t[:], u_dd[blk * P:(blk + 1) * P, :])
                    leb = pw.tile([P, N], F32, tag="wslot")
                    nc.vector.tensor_scalar(leb[:], dens_bc[:],
                                            dens[:, blk:blk + 1], BIG,
                                            op0=OP.is_le, op1=OP.mult)
                    nc.vector.tensor_tensor(leb[:], u_t[:], leb[:], OP.add)
                    nc.vector.tensor_reduce(stats_dmin[:, blk:blk + 1], leb[:],
                                            AX.X, OP.min)
                dmin2 = pstat.tile([P, NBLK], F32, tag="dmin2")
                nc.vector.tensor_tensor(dmin2[:], stats_dmin[:], stats_sq[:], OP.add)
                nc.vector.tensor_scalar(dmin2[:], dmin2[:], dmax_bc[:], None,
                                        op0=OP.min)
                nc.vector.tensor_scalar(dmin2[:], dmin2[:], 0.0, None, op0=OP.max)
                dist_min = pstat.tile([P, NBLK], F32, tag="dist_min")
                sqrt_nr(dmin2[:], dist_min[:], "dmin", [P, NBLK])
                nc.vector.tensor_scalar(dist_min[:], dist_min[:], INV_SQRT_C, None,
                                        op0=OP.mult)
                score = pstat.tile([P, NBLK], F32, tag="score")
                nc.vector.tensor_tensor(score[:], dist_min[:], dens[:], OP.mult)

                # ---- phase 5: top-2 ----
                sc_col = pstat.tile([P, 1], F32, tag="sc_col")
                nc.vector.tensor_reduce(sc_col[:], score[:], AX.X, OP.max)
                _, s1_bc = glob_reduce(sc_col, OP.max, "s1")
                cand = pstat.tile([P, NBLK], F32, tag="cand")
                nc.vector.tensor_scalar(cand[:], score[:], s1_bc[:], BIG,
                                        op0=OP.not_equal, op1=OP.mult)
                nc.vector.tensor_tensor(cand[:], cand[:], iota_tc[:], OP.add)
                cand_col = pstat.tile([P, 1], F32, tag="cand_col")
                nc.vector.tensor_reduce(cand_col[:], cand[:], AX.X, OP.min)
                t1_sc, t1_bc = glob_reduce(cand_col, OP.min, "t1")
                pen = pstat.tile([P, NBLK], F32, tag="pen")
                nc.vector.tensor_scalar(pen[:], iota_tc[:], t1_bc[:], -BIG,
                                        op0=OP.is_equal, op1=OP.mult)
                sc2 = pstat.tile([P, NBLK], F32, tag="sc2")
                nc.vector.tensor_tensor(sc2[:], score[:], pen[:], OP.add)
                sc_col2 = pstat.tile([P, 1], F32, tag="sc_col2")
                nc.vector.tensor_reduce(sc_col2[:], sc2[:], AX.X, OP.max)
                _, s2_bc = glob_reduce(sc_col2, OP.max, "s2")
                cand2 = pstat.tile([P, NBLK], F32, tag="cand2")
                nc.vector.tensor_scalar(cand2[:], sc2[:], s2_bc[:], BIG,
                                        op0=OP.not_equal, op1=OP.mult)
                nc.vector.tensor_tensor(cand2[:], cand2[:], iota_tc[:], OP.add)
                cand_col2 = pstat.tile([P, 1], F32, tag="cand_col2")
                nc.vector.tensor_reduce(cand_col2[:], cand2[:], AX.X, OP.min)
                t2_sc, t2_bc = glob_reduce(cand_col2, OP.min, "t2")

                idx2_d = pdram.tile([2], I32, tag="idx2_d")
                t1_i = pstat.tile([1, 1], I32, tag="t1_i")
                nc.vector.tensor_copy(t1_i[:], t1_sc[:])
                nc.sync.dma_start(idx2_d[0:1], t1_i[:, 0])
                t2_i = pstat.tile([1, 1], I32, tag="t2_i")
                nc.vector.tensor_copy(t2_i[:], t2_sc[:])
                nc.sync.dma_start(idx2_d[1:2], t2_i[:, 0])
                idx2_sb = pstat.tile([2, 1], I32, tag="idx2_sb")
                nc.sync.dma_start(idx2_sb[:], idx2_d[:].unsqueeze(-1))

                # ---- phase 6: dm rows (d2 of the two centers), argmin ----
                dm01 = prow.tile([2, N], F32, tag="rowscratch")
                nc.gpsimd.indirect_dma_start(
                    dm01[:], None, u_dd[:],
                    IndirectOffsetOnAxis(ap=idx2_sb[:], axis=0))
                dm_tm = []
                for ci in range(2):
                    dmr_d = pdram.tile([N], F32, tag=f"dmr_d{ci}")
                    nc.sync.dma_start(dmr_d[:].unsqueeze(0), dm01[ci:ci + 1, :])
                    t_ = pstat.tile([P, NBLK], F32, tag=f"dm_tm{ci}")
                    nc.sync.dma_start(t_[:], dmr_d[:].rearrange("(q p) -> p q", p=P))
                    dm_tm.append(t_)
                sq01 = pstat.tile([2, 1], F32, tag="sq01")
                nc.gpsimd.indirect_dma_start(
                    sq01[:], None, sq_row_d[:].unsqueeze(-1),
                    IndirectOffsetOnAxis(ap=idx2_sb[:], axis=0))
                sq_sel = []
                for ci in range(2):
                    sq_d = pdram.tile([1], F32, tag=f"sqd{ci}")
                    nc.sync.dma_start(sq_d[:].unsqueeze(0), sq01[ci:ci + 1, :])
                    bc_ = pstat.tile([P, 1], F32, tag=f"sqbc{ci}")
                    nc.sync.dma_start(bc_[:], sq_d[:].unsqueeze(0).to_broadcast((P, 1)))
                    sq_sel.append(bc_)
                dd_tm = []
                for ci in range(2):
                    d2t = pstat.tile([P, NBLK], F32, tag=f"d2t{ci}")
                    nc.vector.tensor_scalar(d2t[:], dm_tm[ci][:], sq_sel[ci][:], 0.0,
                                            op0=OP.add, op1=OP.max)
                    ddt = pstat.tile([P, NBLK], F32, tag=f"ddt{ci}")
                    sqrt_nr(d2t[:], ddt[:], f"dd{ci}", [P, NBLK])
                    nc.vector.tensor_scalar(ddt[:], ddt[:], INV_SQRT_C, None,
                                            op0=OP.mult)
                    dd_tm.append(ddt)
                idxf = pstat.tile([P, NBLK], F32, tag="idxf")
                nc.vector.tensor_tensor(idxf[:], dd_tm[1][:], dd_tm[0][:], OP.is_lt)
                mprd = pstat.tile([P, NBLK], I32, tag="mprd")
                nc.vector.tensor_scalar(mprd[:], iota_tc[:], t1_bc[:], None,
                                        op0=OP.is_equal)
                nc.vector.copy_predicated(idxf[:], mprd[:], zero_tm[:])
                nc.vector.tensor_scalar(mprd[:], iota_tc[:], t2_bc[:], None,
                                        op0=OP.is_equal)
                nc.vector.copy_predicated(idxf[:], mprd[:], one_tm[:])
                idx_i = pstat.tile([P, NBLK], I32, tag="idx_i")
                nc.vector.tensor_copy(idx_i[:], idxf[:])
                nc.sync.dma_start(idx_o[b, :].rearrange("(q p) -> p q", p=P),
                                  idx_i[:])

                # ---- phase 7: weights + merge ----
                mask0 = pstat.tile([P, NBLK], F32, tag="mask0")
                nc.vector.tensor_scalar(mask0[:], idxf[:], 0.0, None, op0=OP.is_equal)
                # interleaved [P, 2*NBLK]: col 2q = w*mask0, 2q+1 = w*mask1
                wm2 = pstat.tile([P, 2 * NBLK], F32, tag="wm2")
                wm2v = wm2[:].rearrange("p (q two) -> p q two", two=2)
                nc.vector.tensor_tensor(wm2v[:, :, 0], w_tm[:], mask0[:], OP.mult)
                nc.vector.tensor_tensor(wm2v[:, :, 1], w_tm[:], idxf[:], OP.mult)
                ps_aw = pps.tile([2, 1], F32, tag="ps_aw")
                for blk in range(NBLK):
                    nc.tensor.matmul(ps_aw[:], wm2[:, 2 * blk:2 * blk + 2],
                                     ones_col[:],
                                     start=(blk == 0), stop=(blk == NBLK - 1))
                aw = pstat.tile([2, 1], F32, tag="aw")
                nc.vector.tensor_scalar(aw[:], ps_aw[:], 1e-6, None, op0=OP.add)
                rec = pstat.tile([2, 1], F32, tag="rec")
                nc.vector.reciprocal(rec[:], aw[:])
                rec_d = pdram.tile([2], F32, tag="rec_d")
                nc.sync.dma_start(rec_d[:], rec[:, 0])
                rec0_bc = pstat.tile([P, 1], F32, tag="rec0_bc")
                nc.sync.dma_start(rec0_bc[:], rec_d[0:1].unsqueeze(0).to_broadcast((P, 1)))
                rec1_bc = pstat.tile([P, 1], F32, tag="rec1_bc")
                nc.sync.dma_start(rec1_bc[:], rec_d[1:2].unsqueeze(0).to_broadcast((P, 1)))
                # agg_weight = w * rec[idx]
                rsel = pstat.tile([P, NBLK], F32, tag="rsel")
                nc.vector.tensor_scalar(rsel[:], mask0[:], rec0_bc[:], None, op0=OP.mult)
                rse2 = pstat.tile([P, NBLK], F32, tag="rse2")
                nc.vector.tensor_scalar(rse2[:], idxf[:], rec1_bc[:], None, op0=OP.mult)
                nc.vector.tensor_tensor(rsel[:], rsel[:], rse2[:], OP.add)
                nw_tm = pstat.tile([P, NBLK], F32, tag="nw_tm")
                nc.vector.tensor_tensor(nw_tm[:], w_tm[:], rsel[:], OP.mult)
                nc.sync.dma_start(agg_o[b, :].rearrange("(q p) -> p q", p=P),
                                  nw_tm[:])
                # merge lhsT: nw*mask_c interleaved
                nc.vector.tensor_scalar(wm2v[:, :, 0], wm2v[:, :, 0], rec0_bc[:],
                                        None, op0=OP.mult)
                nc.vector.tensor_scalar(wm2v[:, :, 1], wm2v[:, :, 1], rec1_bc[:],
                                        None, op0=OP.mult)
                ps_m = []
                for hi, h in enumerate(HSP):
                    p_h = pps.tile([2, h], F32, tag=f"ps_cv{hi}")
                    ps_m.append(p_h)
                for blk in range(NBLK):
                    xs_t = pmid.tile([P, C], F32, tag="x_t")
                    nc.sync.dma_start(xs_t[:], xs_dd[blk * P:(blk + 1) * P, :])
                    off = 0
                    for hi, h in enumerate(HSP):
                        nc.tensor.matmul(ps_m[hi][:], wm2[:, 2 * blk:2 * blk + 2],
                                         xs_t[:, off:off + h],
                                         start=(blk == 0), stop=(blk == NBLK - 1))
                        off += h
                xm_sb = pstat.tile([2, C], F32, tag="xm_sb")
                off = 0
                for hi, h in enumerate(HSP):
                    nc.vector.tensor_copy(xm_sb[:, off:off + h], ps_m[hi][:])
                    off += h
                nc.sync.dma_start(xm_o[b, :, :], xm_sb[:])

    nc.compile()
    return nc


# ---------------------------------------------------------------------------
_PROG_CACHE = {}


def _get_program(n_cores, b_per_core, N, C):
    key = (n_cores, b_per_core, N, C)
    if key not in _PROG_CACHE:
        _PROG_CACHE[key] = build_program(n_cores, b_per_core, N, C)
    return _PROG_CACHE[key]


def run_kernel(x, conv_w, ln_gamma, ln_beta, score_w, score_b, noise,
               n_cores=8, trace=False):
    x = np.ascontiguousarray(np.asarray(x, dtype=np.float32))
    conv_w = np.asarray(conv_w, dtype=np.float32)
    B, N, C = x.shape
    assert B % n_cores == 0
    bpc = B // n_cores
    nc = _get_program(n_cores, bpc, N, C)

    NBLK = N // P
    wt = np.ascontiguousarray(np.transpose(conv_w, (1, 2, 0)).reshape(C, 3 * C))
    ident = np.eye(P, dtype=np.float32)
    iota_tc = (np.arange(P)[:, None] + P * np.arange(NBLK)[None, :]).astype(np.float32)
    shared = {
        "wt": wt,
        "gam": np.ascontiguousarray(np.asarray(ln_gamma, np.float32)),
        "bet": np.ascontiguousarray(np.asarray(ln_beta, np.float32)),
        "sw": np.ascontiguousarray(np.asarray(score_w, np.float32).reshape(C)),
        "sb": np.ascontiguousarray(np.asarray(score_b, np.float32).reshape(1)),
        "ident": ident, "identi": ident.astype(np.int32), "iota_tc": iota_tc,
    }
    noise = np.ascontiguousarray(np.asarray(noise, np.float32))
    in_maps = []
    for cid in range(n_cores):
        m = dict(shared)
        m["x"] = np.ascontiguousarray(x[cid * bpc:(cid + 1) * bpc])
        m["noise"] = np.ascontiguousarray(noise[cid * bpc:(cid + 1) * bpc])
        in_maps.append(m)

    res = run_bass_kernel_spmd(nc, in_maps, core_ids=list(range(n_cores)),
                               trace=trace)
    xm = np.concatenate([r["xm"] for r in res.results], axis=0)
    agg = np.concatenate([r["agg"] for r in res.results], axis=0)
    idx = np.concatenate([r["idx"] for r in res.results], axis=0)
    out = (xm, agg.reshape(B, N, 1), idx.astype(np.int32))
    if trace:
        return out, res
    return out


def kernel(x, conv_w, ln_gamma, ln_beta, score_w, score_b, noise):
    return run_kernel(x, conv_w, ln_gamma, ln_beta, score_w, score_b, noise)
